# revision 6
# baseline (speedup 1.0000x reference)
"""GCN layer (out = A_sparse @ (X @ W.T)) on 8 Trainium2 NeuronCores.

Strategy (dest-sharded, no collectives):
  - Shard destination nodes across 8 cores (6250 each), replicate X and W.
  - Compute A@X first (gather + segment-sum), then multiply by W.T per
    dest tile: out = (A @ X) @ W.T.
  - Edges are sorted by (dest tile, src bank, dest window) on host and
    packed densely into 128-edge chunks per (tile, bank) -- chunks are
    NOT aligned to dest windows; a chunk whose edges straddle a window
    boundary simply gets one matmul per touched window, each with its
    own one-hot slice (onehot[e, d] = A_vals[e] * 1[localdest(e) == d],
    OHW=32 wide, prebuilt on host, streamed per 8-tile supergroup).
  - Per chunk the device dma_gather's the 128 source rows of X (256B
    bf16 rows) into SBUF msgs [128 edges, 128 feat]; TensorE multiplies
    msgs.T @ onehot accumulating into PSUM AXT[feat, dest] (start on the
    tile's first matmul zeroes the whole 2KB PSUM zero region).
  - Per dest tile: AXT -> SBUF (bf16), one bf16 matmul with W.T ->
    out[dest, feat]; out rows are staged and stored per 8-tile group.
  - dma_gather indices are int16, so sources split into a "lo" bank
    (rows [0, 32768)) and a "hi" bank (rows [17232, 50000), idx
    src-17232).  Sub-gathers of <=7 chunks keep single_packet legal.
  - SPMD: one program for all 8 cores; per-(tile, bank) chunk counts are
    the max over cores.  Per-core shortfall is padded with idx-0 edges
    up to a shared 64-quantized watermark and idx=-1 beyond it; the
    shared num_idxs_reg per gather op skips the -1 tail entirely (no
    descriptors, no bytes).  Msg buffers are memset once at startup so
    the skipped rows stay finite (their one-hot columns are zero).
"""

import re

import numpy as np

import concourse.bacc as bacc
import concourse.bass as bass
import concourse.mybir as mybir
import concourse.tile as tile
from bass_rust import ScopedClock, VectorClock
from concourse.bass_utils import run_bass_kernel_spmd

N_NODES = 50000
N_EDGES = 1600000
FEAT = 128
N_CORES = 8
NPC = N_NODES // N_CORES  # 6250 dest nodes per core
CH = 128  # edges per chunk
TILE_D = 128  # dests per tile
TPC = (NPC + TILE_D - 1) // TILE_D  # 49 dest tiles per core
OUT_ROWS = TPC * TILE_D  # 6272 padded out rows per core
LO = 32768  # lo bank: src in [0, 32768)
HIB = N_NODES - 32768  # 17232; hi bank rows [HIB, N), idx = src - HIB
OHW = 32  # one-hot width: each matmul covers one 32-dest window
WPT = TILE_D // OHW  # 4 windows per dest tile
SUB = 7  # chunks per dma_gather (56 descs/engine keeps single_packet legal)
OTG = 8  # tiles per OH-load / OUT-store supergroup

FP32 = mybir.dt.float32
BF16 = mybir.dt.bfloat16
I16 = mybir.dt.int16


class SplitDrainTileContext(tile.TileContext):
    """This walrus build allows only one sync-wait on the CTRL_NO drain
    instruction; split the end-of-kernel drain waits across SP nops."""

    def _drain_and_barrier(self, tick_clock, wait_clock):
        gc = tick_clock.global_clock
        vals = [int(x) for x in re.findall(r"-?\d+", repr(gc))]
        for i, v in enumerate(vals):
            if v > 0:
                single = [0] * len(vals)
                single[i] = v
                nopi = self.nc.sync.nop(nofuse=True)
                wait_clock.add_sem_waits(
                    nopi.ins, ScopedClock({None: VectorClock(single)})
                )
        self.nc.sync.drain()
        self.nc.all_engine_barrier()
        assert self.sems is not None
        popped = self.nc._tile_sem_poison_stack.pop()
        assert popped is self._sem_poison
        self.nc.clear_and_free_semaphores(list(self.sems.allocated().values()))
        self.nc.all_engine_barrier()


def _cdiv(a, b):
    return -(-a // b)


def preprocess(X, W, A_vals, A_rows, A_cols):
    """Sort/pad edges, build per-core gather-index and onehot arrays.

    Returns (in_maps, plan) where plan parameterizes the SPMD program:
      plan["ncl"], plan["nchi"]: per-tile lo/hi chunk counts
      plan["mm"][t]: list of (local_chunk, window, oh_slot) matmuls
      plan["regs"]: per-(tile,bank) list of per-op num_idxs values
      plan["nslot"]: total oh slots
    """
    import ml_dtypes
    X = np.ascontiguousarray(np.asarray(X, dtype=np.float32).astype(ml_dtypes.bfloat16))
    W = np.asarray(W, dtype=np.float32)
    vals = np.asarray(A_vals, dtype=np.float32)
    dest = np.asarray(A_rows, dtype=np.int64)
    src = np.asarray(A_cols, dtype=np.int64)

    c = dest // NPC
    r = dest - c * NPC
    t = r // TILE_D
    ld = r - t * TILE_D
    w = ld // OHW
    b = (src >= LO).astype(np.int64)
    # sort by (core, tile, bank, window); edges pack densely per bank run
    g = ((c * TPC + t) * 2 + b) * WPT + w
    order = np.argsort(g, kind="stable")
    g_s = g[order]
    c_s = c[order]
    ld_s = ld[order]
    src_s = src[order]
    b_s = b[order]
    val_s = vals[order]

    ngroups = N_CORES * TPC * 2 * WPT
    counts = np.bincount(g_s, minlength=ngroups)
    cnt = counts.reshape(N_CORES, TPC, 2, WPT)
    cnt_tb = cnt.sum(axis=3)  # [NC, TPC, 2] edges per (core, tile, bank)
    maxc_tb = cnt_tb.max(axis=0)  # [TPC, 2]
    # every tile needs >= 1 chunk so its PSUM region is written
    forced = maxc_tb.sum(axis=1) == 0
    maxc_eff = maxc_tb.copy()
    maxc_eff[forced, 0] = CH
    chunks_tb = _cdiv(maxc_eff, CH)  # [TPC, 2]
    chunks_tb[maxc_eff == 0] = 0
    # shared per-core idx watermark: pads in [count, wm) are idx 0 (real
    # descriptors), [wm, chunks*CH) are idx -1 (skipped); wm is maxc
    # rounded up to 64 to bound the distinct num_idxs_reg values
    wm_tb = np.minimum(_cdiv(maxc_eff, 64) * 64, chunks_tb * CH)  # [TPC, 2]
    ncl = chunks_tb[:, 0]
    nchi = chunks_tb[:, 1]
    nch = ncl + nchi
    TC = int(nch.sum())

    # chunk base of each (t, b) bank run within the core's chunk array
    flat_chunks = chunks_tb.reshape(-1)  # [TPC*2]
    bank_ch0 = np.zeros(TPC * 2, np.int64)
    bank_ch0[1:] = np.cumsum(flat_chunks)[:-1]
    bank_ch0 = bank_ch0.reshape(TPC, 2)

    # per-core slot of each edge inside its (t, b) bank run (w-sorted)
    group_start = np.zeros(ngroups, np.int64)
    group_start[1:] = np.cumsum(counts)[:-1]
    pos_in_w = np.arange(len(g_s), dtype=np.int64) - group_start[g_s]
    # start of each (c,t,b,w) group within its bank run
    wstart = np.zeros((N_CORES, TPC, 2, WPT), np.int64)
    wstart[:, :, :, 1:] = np.cumsum(cnt, axis=3)[:, :, :, :-1]
    slot_in_bank = wstart[c_s, t[order], b_s, w[order]] + pos_in_w
    abs_chunk = bank_ch0[t[order], b_s] + slot_in_bank // CH
    abs_slot = abs_chunk * CH + (slot_in_bank % CH)
    idx_val = np.where(b_s == 0, src_s, src_s - HIB).astype(np.int16)

    # matmul plan: per (t, b, w) the chunk span is the union over cores
    wend = wstart + cnt  # exclusive ends
    w_lo = wstart.min(axis=0)  # [TPC, 2, WPT]
    w_hi = wend.max(axis=0)
    mm = [[] for _ in range(TPC)]
    slot_of = {}
    nslot = 0
    oh_t0 = np.zeros(TPC + 1, np.int64)  # first oh slot of each tile
    for ti in range(TPC):
        for bi in range(2):
            base = 0 if bi == 0 else int(ncl[ti])
            for wi in range(WPT):
                lo_, hi_ = int(w_lo[ti, bi, wi]), int(w_hi[ti, bi, wi])
                if hi_ <= lo_:
                    continue
                for ck in range(lo_ // CH, _cdiv(hi_, CH)):
                    mm[ti].append((base + ck, wi))
                    slot_of[(ti, bi, ck, wi)] = nslot
                    nslot += 1
        if not mm[ti]:  # forced empty tile: one zero matmul
            mm[ti].append((0, 0))
            slot_of[(ti, 0, 0, 0)] = nslot
            nslot += 1
        oh_t0[ti + 1] = nslot
    # attach oh slots to the per-tile matmul list
    for ti in range(TPC):
        lst = mm[ti]
        mm[ti] = []
        for k, (lc, wi) in enumerate(lst):
            bi = 0 if lc < int(ncl[ti]) else 1
            ck = lc if bi == 0 else lc - int(ncl[ti])
            mm[ti].append((lc, wi, slot_of[(ti, bi, ck, wi)]))

    # oh slot id per edge (vectorized lookup via encoded keys)
    enc = lambda tt, bb, cc, ww: ((tt * 2 + bb) * 64 + cc) * WPT + ww
    keys = np.array(sorted(slot_of, key=lambda k: slot_of[k]))
    key_codes = enc(keys[:, 0], keys[:, 1], keys[:, 2], keys[:, 3])
    key_order = np.argsort(key_codes)
    sorted_codes = key_codes[key_order]
    sorted_slots = np.arange(nslot)[key_order]
    edge_codes = enc(t[order], b_s, slot_in_bank // CH, w[order])
    edge_slot = sorted_slots[np.searchsorted(sorted_codes, edge_codes)]

    # per-(t, b) gather op num_idxs values (shared across cores)
    regs = {}
    for ti in range(TPC):
        for bi in range(2):
            nchunks = int(chunks_tb[ti, bi])
            wm = int(wm_tb[ti, bi])
            ops = []
            a = 0
            while a < nchunks:
                n = min(SUB, nchunks - a)
                reg = max(0, min(wm - a * CH, n * CH))
                assert reg > 0
                ops.append((n, reg))
                a += n
            regs[(ti, bi)] = ops

    TCE = TC * CH
    in_maps = []
    WT = np.ascontiguousarray(W.T.astype(ml_dtypes.bfloat16))  # [in, out]
    # -1 template: slots in [wm, chunks*CH) of each bank run
    idx_template = np.zeros(TCE, np.int16)
    for ti in range(TPC):
        for bi in range(2):
            s0 = int(bank_ch0[ti, bi]) * CH
            idx_template[s0 + int(wm_tb[ti, bi]) : s0 + int(chunks_tb[ti, bi]) * CH] = -1
    for core in range(N_CORES):
        m = c_s == core
        fl = abs_slot[m]
        idx_flat = idx_template.copy()
        idx_flat[fl] = idx_val[m]
        idx_w = np.ascontiguousarray(idx_flat.reshape(TCE // 16, 16).T)
        idx_rep = np.ascontiguousarray(np.tile(idx_w, (8, 1)))  # [128, TCE/16]
        oh = np.zeros((CH, nslot, OHW), ml_dtypes.bfloat16)
        oh[fl % CH, edge_slot[m], ld_s[m] % OHW] = val_s[m].astype(ml_dtypes.bfloat16)
        in_maps.append({"X": X, "WT": WT, "OH": oh, "IDX": idx_rep})

    plan = {
        "ncl": [int(x) for x in ncl],
        "nchi": [int(x) for x in nchi],
        "mm": mm,
        "regs": regs,
        "nslot": nslot,
        "oh_t0": [int(x) for x in oh_t0],
    }
    return in_maps, plan


def build_program(plan):
    """Emit the SPMD Bass program."""
    ncl, nchi = plan["ncl"], plan["nchi"]
    mm, regs, nslot, oh_t0 = plan["mm"], plan["regs"], plan["nslot"], plan["oh_t0"]
    nch = [l + h for l, h in zip(ncl, nchi)]
    TC = sum(nch)
    nch_max = max(nch)
    tile_ch0 = np.zeros(TPC, np.int64)
    tile_ch0[1:] = np.cumsum(nch)[:-1]
    ngrp = _cdiv(TPC, OTG)
    # max oh slots per supergroup (for pool sizing)
    grp_slots = [oh_t0[min(TPC, (gi + 1) * OTG)] - oh_t0[gi * OTG] for gi in range(ngrp)]
    gsl_max = max(grp_slots)

    nc = bacc.Bacc("TRN2", target_bir_lowering=False, debug=False, num_swdge_queues=4, dynamic_dma_scratch_size=65536)
    X = nc.dram_tensor("X", [N_NODES, FEAT], BF16, kind="ExternalInput")
    WT = nc.dram_tensor("WT", [FEAT, FEAT], BF16, kind="ExternalInput")
    OH = nc.dram_tensor("OH", [CH, nslot, OHW], BF16, kind="ExternalInput")
    IDX = nc.dram_tensor("IDX", [128, TC * CH // 16], I16, kind="ExternalInput")
    OUT = nc.dram_tensor("OUT", [OUT_ROWS, FEAT], FP32, kind="ExternalOutput")

    x_lo = X[0:LO, :]
    x_hi = X[HIB:N_NODES, :]

    # strict round-robin across the 4 SWDGE queues keeps Tile's DMASW sem
    # lanes (rotating mod 8) each locked to one queue
    qctr = [0]

    def pick_queue():
        q = qctr[0] % 4
        qctr[0] += 1
        return q

    with SplitDrainTileContext(nc) as tc:
        with (
            tc.tile_pool(name="const", bufs=1) as const_pool,
            tc.tile_pool(name="oh", bufs=2) as oh_pool,
            tc.tile_pool(name="msg", bufs=3) as msg_pool,
            tc.tile_pool(name="axt", bufs=2) as axt_pool,
            tc.tile_pool(name="outp", bufs=2) as out_pool,
            tc.tile_pool(name="ps_axt", bufs=2, space="PSUM") as ps_axt_pool,
            tc.tile_pool(name="ps_out", bufs=2, space="PSUM") as ps_out_pool,
        ):
            reg_cache = {}

            def nreg(v):
                if v not in reg_cache:
                    reg_cache[v] = nc.gpsimd.to_reg(v)
                return reg_cache[v]

            wt_sb = const_pool.tile([FEAT, FEAT], BF16, tag="wt")
            nc.sync.dma_start(wt_sb[:], WT[:])
            idx_sb = const_pool.tile([128, TC * CH // 16], I16, tag="idx")
            nc.sync.dma_start(idx_sb[:], IDX[:])
            # zero all msg buffers once: rows the -1 idx tail skips stay
            # finite (their one-hot columns are zero)
            for _ in range(3):
                mz = msg_pool.tile([CH, nch_max, FEAT], BF16, tag="msg")
                nc.vector.memset(mz[:], 0)

            for gi in range(ngrp):
                tlo, thi = gi * OTG, min(TPC, (gi + 1) * OTG)
                sl0, sl1 = oh_t0[tlo], oh_t0[thi]
                oh_g = oh_pool.tile([CH, gsl_max * OHW], BF16, tag="oh")
                nc.sync.dma_start(oh_g[:, : (sl1 - sl0) * OHW], OH[:, sl0:sl1, :])
                out_g = out_pool.tile([TILE_D, (thi - tlo) * FEAT], FP32, tag="out")
                for t in range(tlo, thi):
                    ch0 = int(tile_ch0[t])
                    nl, nt = ncl[t], nch[t]
                    msg_t = msg_pool.tile([CH, nch_max, FEAT], BF16, tag="msg")
                    for bi, (c0, srcb) in enumerate(((0, x_lo), (nl, x_hi))):
                        a = 0
                        for n, reg in regs[(t, bi)]:
                            nc.gpsimd.dma_gather(
                                msg_t[:, c0 + a : c0 + a + n, :],
                                srcb,
                                idx_sb[:, 8 * (ch0 + c0 + a) : 8 * (ch0 + c0 + a + n)],
                                n * CH,
                                nreg(reg),
                                FEAT,
                                elem_step=FEAT,
                                single_packet=True,
                                queue_num=pick_queue(),
                            )
                            a += n
                    ps_axt = ps_axt_pool.tile([FEAT, TILE_D], FP32, tag="psa")
                    nmm = len(mm[t])
                    for k, (lc, wi, sl) in enumerate(mm[t]):
                        lsl = sl - sl0
                        nc.tensor.matmul(
                            ps_axt[:, wi * OHW : (wi + 1) * OHW],
                            msg_t[:, lc, :],
                            oh_g[:, lsl * OHW : (lsl + 1) * OHW],
                            start=(k == 0),
                            stop=(k == nmm - 1),
                        )
                    axt = axt_pool.tile([FEAT, TILE_D], BF16, tag="axt")
                    nc.vector.tensor_copy(axt[:], ps_axt[:])
                    ps_out = ps_out_pool.tile([TILE_D, FEAT], FP32, tag="pso")
                    nc.tensor.matmul(ps_out[:], axt[:], wt_sb[:], start=True, stop=True)
                    nc.vector.tensor_copy(
                        out_g[:, (t - tlo) * FEAT : (t - tlo + 1) * FEAT], ps_out[:]
                    )
                ot = (thi - tlo) * TILE_D
                nc.sync.dma_start(
                    OUT[tlo * TILE_D : tlo * TILE_D + ot, :].rearrange(
                        "(j p) f -> p j f", p=TILE_D
                    ),
                    out_g[:].rearrange("p (j f) -> p j f", f=FEAT),
                )
    nc.compile()
    return nc


def _ensure_ntff_hook():
    """The agent image's antenv lacks axon_hooks; recreate it and register
    the ctypes NTFF profiling hook the axon boot would have installed."""
    try:
        from antenv import axon_hooks  # noqa: F401

        return
    except ImportError:
        pass
    import sys
    import types

    import antenv

    mod = types.ModuleType("antenv.axon_hooks")
    state = {"hook": None}
    mod.set_axon_ntff_profile_hook = lambda h: state.__setitem__("hook", h)
    mod.get_axon_ntff_profile_hook = lambda: state["hook"]
    sys.modules["antenv.axon_hooks"] = mod
    antenv.axon_hooks = mod
    try:
        from trn_agent_boot.trn_boot import _ntff_profile_via_ctypes

        mod.set_axon_ntff_profile_hook(
            _ntff_profile_via_ctypes("/opt/axon/libaxon_pjrt.so")
        )
    except Exception:
        pass


def _run(inputs, trace=False, trace_kwargs=None):
    if trace:
        _ensure_ntff_hook()
    in_maps, plan = preprocess(
        inputs["X"], inputs["W"], inputs["A_vals"], inputs["A_rows"], inputs["A_cols"]
    )
    nc = build_program(plan)
    res = run_bass_kernel_spmd(
        nc,
        in_maps,
        list(range(N_CORES)),
        trace=trace,
        **(trace_kwargs or {}),
    )
    out = np.concatenate(
        [res.results[i]["OUT"][:NPC] for i in range(N_CORES)], axis=0
    )
    return out.astype(np.float32, copy=False), res


def kernel(X, W, A_vals, A_rows, A_cols):
    out, _ = _run(
        {"X": X, "W": W, "A_vals": A_vals, "A_rows": A_rows, "A_cols": A_cols}
    )
    return out


def kernel_traced(X, W, A_vals, A_rows, A_cols):
    """Like kernel() but profiles on HW; returns (out, exec_time_ns)."""
    out, res = _run(
        {"X": X, "W": W, "A_vals": A_vals, "A_rows": A_rows, "A_cols": A_cols},
        trace=True,
        trace_kwargs={"trace_cores": list(range(N_CORES))},
    )
    return out, res.exec_time_ns


# revision 9
# speedup vs baseline: 1.1766x; 1.1766x over previous
"""GCN layer (out = A_sparse @ (X @ W.T)) on 8 Trainium2 NeuronCores.

Strategy (dest-sharded, no collectives):
  - Shard destination nodes across 8 cores (6250 each), replicate X and W.
  - Compute A@X first (gather + segment-sum), then multiply by W.T per
    dest tile: out = (A @ X) @ W.T.
  - Edges are sorted by (4-tile supergroup, src bank, tile, dest window)
    on host and packed densely into 128-edge chunks per (tile, bank);
    chunks are NOT aligned to dest windows -- a chunk whose edges
    straddle a window boundary gets one matmul per touched window, each
    with its own one-hot slice (onehot[e, d] = A_vals[e] *
    1[localdest(e) == d], OHW=32 wide, prebuilt on host, streamed per
    supergroup).
  - Gathers run per (supergroup, bank): the 4 tiles' chunks form one
    contiguous run split into <=7-chunk dma_gather ops (56 descs/engine
    keeps single_packet legal), emitted largest-first so the strict
    queue rotation balances descriptor load across the 4 SWDGE queues.
  - Per chunk the device dma_gather's the 128 source rows of X (256B
    bf16 rows) into SBUF msgs [128 edges, 128 feat]; TensorE multiplies
    msgs.T @ onehot accumulating into PSUM AXT[feat, dest] (start on the
    tile's first matmul zeroes the whole 2KB PSUM zero region).
  - Per dest tile: AXT -> SBUF (bf16), one bf16 matmul with W.T ->
    out[dest, feat]; out rows are staged and stored per supergroup.
  - dma_gather indices are int16, so sources split into a "lo" bank
    (rows [0, 32768)) and a "hi" bank (rows [17232, 50000), idx
    src-17232).
  - SPMD: one program for all 8 cores; per-(tile, bank) chunk counts are
    the max over cores.  Per-core shortfall pads with idx-0 edges; the
    tail of each (supergroup, bank) run is idx=-1 beyond a shared
    64-quantized watermark and the op's num_idxs_reg skips it (no
    descriptors, no bytes).  Msg buffers are memset once at startup so
    skipped rows stay finite (their one-hot columns are zero).
"""

import re

import numpy as np

import concourse.bacc as bacc
import concourse.bass as bass
import concourse.mybir as mybir
import concourse.tile as tile
from bass_rust import ScopedClock, VectorClock
from concourse.bass_utils import run_bass_kernel_spmd

N_NODES = 50000
N_EDGES = 1600000
FEAT = 128
N_CORES = 8
NPC = N_NODES // N_CORES  # 6250 dest nodes per core
CH = 128  # edges per chunk
TILE_D = 128  # dests per tile
TPC = (NPC + TILE_D - 1) // TILE_D  # 49 dest tiles per core
OUT_ROWS = TPC * TILE_D  # 6272 padded out rows per core
LO = 32768  # lo bank: src in [0, 32768)
HIB = N_NODES - 32768  # 17232; hi bank rows [HIB, N), idx = src - HIB
OHW = 32  # one-hot width: each matmul covers one 32-dest window
WPT = TILE_D // OHW  # 4 windows per dest tile
SUB = 7  # chunks per dma_gather (56 descs/engine keeps single_packet legal)
GT = 4  # tiles per supergroup (msg/oh/out batching + gather run packing)
NG = (TPC + GT - 1) // GT  # 13 supergroups

FP32 = mybir.dt.float32
BF16 = mybir.dt.bfloat16
I16 = mybir.dt.int16


class SplitDrainTileContext(tile.TileContext):
    """This walrus build allows only one sync-wait on the CTRL_NO drain
    instruction; split the end-of-kernel drain waits across SP nops."""

    def _drain_and_barrier(self, tick_clock, wait_clock):
        gc = tick_clock.global_clock
        vals = [int(x) for x in re.findall(r"-?\d+", repr(gc))]
        for i, v in enumerate(vals):
            if v > 0:
                single = [0] * len(vals)
                single[i] = v
                nopi = self.nc.sync.nop(nofuse=True)
                wait_clock.add_sem_waits(
                    nopi.ins, ScopedClock({None: VectorClock(single)})
                )
        self.nc.sync.drain()
        self.nc.all_engine_barrier()
        assert self.sems is not None
        popped = self.nc._tile_sem_poison_stack.pop()
        assert popped is self._sem_poison
        self.nc.clear_and_free_semaphores(list(self.sems.allocated().values()))
        self.nc.all_engine_barrier()


def _cdiv(a, b):
    return -(-a // b)


def preprocess(X, W, A_vals, A_rows, A_cols):
    """Sort/pad edges, build per-core gather-index and onehot arrays."""
    import ml_dtypes
    X = np.ascontiguousarray(np.asarray(X, dtype=np.float32).astype(ml_dtypes.bfloat16))
    W = np.asarray(W, dtype=np.float32)
    vals = np.asarray(A_vals, dtype=np.float32)
    dest = np.asarray(A_rows, dtype=np.int64)
    src = np.asarray(A_cols, dtype=np.int64)

    c = dest // NPC
    r = dest - c * NPC
    t = r // TILE_D
    ld = r - t * TILE_D
    w = ld // OHW
    b = (src >= LO).astype(np.int64)
    gi = t // GT
    ti = t - gi * GT
    # sort by (core, supergroup, bank, tile-in-group, window)
    g = (((c * NG + gi) * 2 + b) * GT + ti) * WPT + w
    order = np.argsort(g, kind="stable")
    g_s = g[order]
    c_s = c[order]
    ld_s = ld[order]
    src_s = src[order]
    b_s = b[order]
    t_s = t[order]
    w_s = w[order]
    val_s = vals[order]

    ngroups = N_CORES * NG * 2 * GT * WPT
    counts = np.bincount(g_s, minlength=ngroups)
    cnt = counts.reshape(N_CORES, NG, 2, GT, WPT)
    cnt_tb = cnt.sum(axis=4)  # [NC, NG, 2, GT] edges per (core, tile, bank)
    maxc_gbt = cnt_tb.max(axis=0)  # [NG, 2, GT]
    # every tile needs >= 1 chunk so its PSUM region is written
    tile_tot = maxc_gbt.sum(axis=1)  # [NG, GT]
    forced = tile_tot == 0
    maxc_eff = maxc_gbt.copy()
    maxc_eff[:, 0, :][forced] = CH
    chunks_gbt = _cdiv(maxc_eff, CH)  # [NG, 2, GT]
    # kill chunk slots for tiles beyond TPC in the last group
    for gi_ in range(NG):
        for ti_ in range(GT):
            if gi_ * GT + ti_ >= TPC:
                chunks_gbt[gi_, :, ti_] = 0
                maxc_eff[gi_, :, ti_] = 0

    # chunk layout: per group: [lo: t0..t3 | hi: t0..t3]
    sec_chunks = chunks_gbt.sum(axis=2)  # [NG, 2] chunks per section
    grp_chunks = sec_chunks.sum(axis=1)  # [NG]
    grp_ch0 = np.zeros(NG, np.int64)
    grp_ch0[1:] = np.cumsum(grp_chunks)[:-1]
    TC = int(grp_chunks.sum())
    # block (chunk base) of each (t, b) within the global chunk array
    block_ch0 = np.zeros((TPC, 2), np.int64)
    for gi_ in range(NG):
        off = int(grp_ch0[gi_])
        for bi_ in range(2):
            for ti_ in range(GT):
                t_ = gi_ * GT + ti_
                if t_ >= TPC:
                    continue
                block_ch0[t_, bi_] = off
                off += int(chunks_gbt[gi_, bi_, ti_])

    # watermark for the -1 tail: only the LAST tile (with chunks) of each
    # (group, bank) section can trim; interior pads are idx-0 gathers
    wm_gb = np.zeros((NG, 2), np.int64)  # nonneg idx count per section
    for gi_ in range(NG):
        for bi_ in range(2):
            sec = int(sec_chunks[gi_, bi_])
            if sec == 0:
                continue
            # last tile in this section with chunks
            last_ti = max(
                ti_ for ti_ in range(GT) if chunks_gbt[gi_, bi_, ti_] > 0
            )
            head = sec - int(chunks_gbt[gi_, bi_, last_ti])
            wm_last = min(
                _cdiv(int(maxc_eff[gi_, bi_, last_ti]), 64) * 64,
                int(chunks_gbt[gi_, bi_, last_ti]) * CH,
            )
            wm_gb[gi_, bi_] = head * CH + wm_last

    # per-core slot of each edge inside its (t, b) tile-block (w-sorted)
    group_start = np.zeros(ngroups, np.int64)
    group_start[1:] = np.cumsum(counts)[:-1]
    pos_in_w = np.arange(len(g_s), dtype=np.int64) - group_start[g_s]
    wstart = np.zeros((N_CORES, NG, 2, GT, WPT), np.int64)
    wstart[..., 1:] = np.cumsum(cnt, axis=4)[..., :-1]
    gi_s = t_s // GT
    ti_s = t_s - gi_s * GT
    slot_in_block = wstart[c_s, gi_s, b_s, ti_s, w_s] + pos_in_w
    abs_chunk = block_ch0[t_s, b_s] + slot_in_block // CH
    abs_slot = abs_chunk * CH + (slot_in_block % CH)
    idx_val = np.where(b_s == 0, src_s, src_s - HIB).astype(np.int16)

    # matmul plan: per (t, b, w) the chunk span is the union over cores
    wend = wstart + cnt
    w_lo = wstart.min(axis=0)  # [NG, 2, GT, WPT]
    w_hi = wend.max(axis=0)
    mm = [[] for _ in range(TPC)]
    slot_of = {}
    nslot = 0
    oh_g0 = np.zeros(NG + 1, np.int64)  # first oh slot of each group
    for gi_ in range(NG):
        for ti_ in range(GT):
            t_ = gi_ * GT + ti_
            if t_ >= TPC:
                continue
            for bi_ in range(2):
                for wi_ in range(WPT):
                    lo_ = int(w_lo[gi_, bi_, ti_, wi_])
                    hi_ = int(w_hi[gi_, bi_, ti_, wi_])
                    if hi_ <= lo_:
                        continue
                    for ck in range(lo_ // CH, _cdiv(hi_, CH)):
                        lc = int(block_ch0[t_, bi_]) - int(grp_ch0[gi_]) + ck
                        mm[t_].append((lc, wi_, nslot))
                        slot_of[(t_, bi_, ck, wi_)] = nslot
                        nslot += 1
            if not mm[t_]:  # forced empty tile: one zero matmul
                lc = int(block_ch0[t_, 0]) - int(grp_ch0[gi_])
                mm[t_].append((lc, 0, nslot))
                slot_of[(t_, 0, 0, 0)] = nslot
                nslot += 1
        oh_g0[gi_ + 1] = nslot

    # gather ops per (group, bank): <=7-chunk splits with trailing trim
    regs = {}
    for gi_ in range(NG):
        for bi_ in range(2):
            sec = int(sec_chunks[gi_, bi_])
            wm = int(wm_gb[gi_, bi_])
            ops = []
            a = 0
            while a < sec:
                n = min(SUB, sec - a)
                reg = max(0, min(wm - a * CH, n * CH))
                assert reg > 0, (gi_, bi_, a)
                ops.append((a, n, reg))
                a += n
            regs[(gi_, bi_)] = ops

    TCE = TC * CH
    in_maps = []
    WT = np.ascontiguousarray(W.T.astype(ml_dtypes.bfloat16))
    idx_template = np.zeros(TCE, np.int16)
    for gi_ in range(NG):
        for bi_ in range(2):
            sec = int(sec_chunks[gi_, bi_])
            if sec == 0:
                continue
            s0 = (int(grp_ch0[gi_]) + (int(sec_chunks[gi_, 0]) if bi_ else 0)) * CH
            idx_template[s0 + int(wm_gb[gi_, bi_]) : s0 + sec * CH] = -1

    # oh slot id per edge (vectorized lookup)
    enc = lambda tt, bb, cc, ww: ((tt * 2 + bb) * 64 + cc) * WPT + ww
    keys = np.array(sorted(slot_of, key=lambda k: slot_of[k]))
    key_codes = enc(keys[:, 0], keys[:, 1], keys[:, 2], keys[:, 3])
    key_order = np.argsort(key_codes)
    sorted_codes = key_codes[key_order]
    sorted_slots = np.arange(nslot)[key_order]
    edge_codes = enc(t_s, b_s, slot_in_block // CH, w_s)
    edge_slot = sorted_slots[np.searchsorted(sorted_codes, edge_codes)]

    for core in range(N_CORES):
        m = c_s == core
        fl = abs_slot[m]
        idx_flat = idx_template.copy()
        idx_flat[fl] = idx_val[m]
        idx_w = np.ascontiguousarray(idx_flat.reshape(TCE // 16, 16).T)
        idx_rep = np.ascontiguousarray(np.tile(idx_w, (8, 1)))
        oh = np.zeros((CH, nslot, OHW), ml_dtypes.bfloat16)
        oh[fl % CH, edge_slot[m], ld_s[m] % OHW] = val_s[m].astype(ml_dtypes.bfloat16)
        in_maps.append({"X": X, "WT": WT, "OH": oh, "IDX": idx_rep})

    plan = {
        "grp_chunks": [int(x) for x in grp_chunks],
        "grp_ch0": [int(x) for x in grp_ch0],
        "sec_lo": [int(sec_chunks[gi_, 0]) for gi_ in range(NG)],
        "mm": mm,
        "regs": regs,
        "nslot": nslot,
        "oh_g0": [int(x) for x in oh_g0],
        "TC": TC,
    }
    return in_maps, plan


def build_program(plan):
    """Emit the SPMD Bass program."""
    mm, regs, nslot, oh_g0 = plan["mm"], plan["regs"], plan["nslot"], plan["oh_g0"]
    grp_chunks, grp_ch0, sec_lo = plan["grp_chunks"], plan["grp_ch0"], plan["sec_lo"]
    TC = plan["TC"]
    gch_max = max(grp_chunks)
    gsl_max = max(oh_g0[gi + 1] - oh_g0[gi] for gi in range(NG))

    nc = bacc.Bacc("TRN2", target_bir_lowering=False, debug=False, num_swdge_queues=4, dynamic_dma_scratch_size=65536)
    X = nc.dram_tensor("X", [N_NODES, FEAT], BF16, kind="ExternalInput")
    WT = nc.dram_tensor("WT", [FEAT, FEAT], BF16, kind="ExternalInput")
    OH = nc.dram_tensor("OH", [CH, nslot, OHW], BF16, kind="ExternalInput")
    IDX = nc.dram_tensor("IDX", [128, TC * CH // 16], I16, kind="ExternalInput")
    OUT = nc.dram_tensor("OUT", [OUT_ROWS, FEAT], FP32, kind="ExternalOutput")

    x_lo = X[0:LO, :]
    x_hi = X[HIB:N_NODES, :]

    qctr = [0]

    def pick_queue():
        q = qctr[0] % 4
        qctr[0] += 1
        return q

    with SplitDrainTileContext(nc) as tc:
        with (
            tc.tile_pool(name="const", bufs=1) as const_pool,
            tc.tile_pool(name="oh", bufs=2) as oh_pool,
            tc.tile_pool(name="msg", bufs=2) as msg_pool,
            tc.tile_pool(name="axt", bufs=2) as axt_pool,
            tc.tile_pool(name="outp", bufs=2) as out_pool,
            tc.tile_pool(name="ps_axt", bufs=2, space="PSUM") as ps_axt_pool,
            tc.tile_pool(name="ps_out", bufs=2, space="PSUM") as ps_out_pool,
        ):
            reg_cache = {}

            def nreg(v):
                if v not in reg_cache:
                    reg_cache[v] = nc.gpsimd.to_reg(v)
                return reg_cache[v]

            wt_sb = const_pool.tile([FEAT, FEAT], BF16, tag="wt")
            nc.sync.dma_start(wt_sb[:], WT[:])
            idx_sb = const_pool.tile([128, TC * CH // 16], I16, tag="idx")
            nc.sync.dma_start(idx_sb[:], IDX[:])
            # zero msg buffers once: rows the -1 idx tail skips stay finite
            for _ in range(2):
                mz = msg_pool.tile([CH, gch_max, FEAT], BF16, tag="msg")
                nc.vector.memset(mz[:], 0)

            for gi in range(NG):
                tlo, thi = gi * GT, min(TPC, (gi + 1) * GT)
                ch0 = grp_ch0[gi]
                sl0, sl1 = oh_g0[gi], oh_g0[gi + 1]
                oh_g = oh_pool.tile([CH, gsl_max * OHW], BF16, tag="oh")
                nc.sync.dma_start(oh_g[:, : (sl1 - sl0) * OHW], OH[:, sl0:sl1, :])
                msg_g = msg_pool.tile([CH, gch_max, FEAT], BF16, tag="msg")
                # gather ops for both bank sections, largest-first so the
                # strict queue rotation balances descriptor load
                ops = []
                for bi, srcb in ((0, x_lo), (1, x_hi)):
                    base = 0 if bi == 0 else sec_lo[gi]
                    for a, n, reg in regs.get((gi, bi), []):
                        ops.append((reg, base + a, n, srcb))
                ops.sort(key=lambda o: -o[0])
                for reg, gpos, n, srcb in ops:
                    nc.gpsimd.dma_gather(
                        msg_g[:, gpos : gpos + n, :],
                        srcb,
                        idx_sb[:, 8 * (ch0 + gpos) : 8 * (ch0 + gpos + n)],
                        n * CH,
                        nreg(reg),
                        FEAT,
                        elem_step=FEAT,
                        single_packet=True,
                        queue_num=pick_queue(),
                    )
                out_g = out_pool.tile([TILE_D, (thi - tlo) * FEAT], FP32, tag="out")
                for t in range(tlo, thi):
                    ps_axt = ps_axt_pool.tile([FEAT, TILE_D], FP32, tag="psa")
                    nmm = len(mm[t])
                    for k, (lc, wi, sl) in enumerate(mm[t]):
                        lsl = sl - sl0
                        nc.tensor.matmul(
                            ps_axt[:, wi * OHW : (wi + 1) * OHW],
                            msg_g[:, lc, :],
                            oh_g[:, lsl * OHW : (lsl + 1) * OHW],
                            start=(k == 0),
                            stop=(k == nmm - 1),
                        )
                    axt = axt_pool.tile([FEAT, TILE_D], BF16, tag="axt")
                    nc.vector.tensor_copy(axt[:], ps_axt[:])
                    ps_out = ps_out_pool.tile([TILE_D, FEAT], FP32, tag="pso")
                    nc.tensor.matmul(ps_out[:], axt[:], wt_sb[:], start=True, stop=True)
                    nc.vector.tensor_copy(
                        out_g[:, (t - tlo) * FEAT : (t - tlo + 1) * FEAT], ps_out[:]
                    )
                ot = (thi - tlo) * TILE_D
                nc.sync.dma_start(
                    OUT[tlo * TILE_D : tlo * TILE_D + ot, :].rearrange(
                        "(j p) f -> p j f", p=TILE_D
                    ),
                    out_g[:].rearrange("p (j f) -> p j f", f=FEAT),
                )
    nc.compile()
    return nc


def _ensure_ntff_hook():
    """The agent image's antenv lacks axon_hooks; recreate it and register
    the ctypes NTFF profiling hook the axon boot would have installed."""
    try:
        from antenv import axon_hooks  # noqa: F401

        return
    except ImportError:
        pass
    import sys
    import types

    import antenv

    mod = types.ModuleType("antenv.axon_hooks")
    state = {"hook": None}
    mod.set_axon_ntff_profile_hook = lambda h: state.__setitem__("hook", h)
    mod.get_axon_ntff_profile_hook = lambda: state["hook"]
    sys.modules["antenv.axon_hooks"] = mod
    antenv.axon_hooks = mod
    try:
        from trn_agent_boot.trn_boot import _ntff_profile_via_ctypes

        mod.set_axon_ntff_profile_hook(
            _ntff_profile_via_ctypes("/opt/axon/libaxon_pjrt.so")
        )
    except Exception:
        pass


def _run(inputs, trace=False, trace_kwargs=None):
    if trace:
        _ensure_ntff_hook()
    in_maps, plan = preprocess(
        inputs["X"], inputs["W"], inputs["A_vals"], inputs["A_rows"], inputs["A_cols"]
    )
    nc = build_program(plan)
    res = run_bass_kernel_spmd(
        nc,
        in_maps,
        list(range(N_CORES)),
        trace=trace,
        **(trace_kwargs or {}),
    )
    out = np.concatenate(
        [res.results[i]["OUT"][:NPC] for i in range(N_CORES)], axis=0
    )
    return out.astype(np.float32, copy=False), res


def kernel(X, W, A_vals, A_rows, A_cols):
    out, _ = _run(
        {"X": X, "W": W, "A_vals": A_vals, "A_rows": A_rows, "A_cols": A_cols}
    )
    return out


def kernel_traced(X, W, A_vals, A_rows, A_cols):
    """Like kernel() but profiles on HW; returns (out, exec_time_ns)."""
    out, res = _run(
        {"X": X, "W": W, "A_vals": A_vals, "A_rows": A_rows, "A_cols": A_cols},
        trace=True,
        trace_kwargs={"trace_cores": list(range(N_CORES))},
    )
    return out, res.exec_time_ns
